# revision 6
# baseline (speedup 1.0000x reference)
"""Trainium2 Bass kernel for GNN message passing (nn_BDLModule_34488587387542).

Computation (N=100000 nodes, E=1600000 edges, DIM=128):
    deg  = out-degree(src);  a = rsqrt(deg)
    h0   = per-node block rotation of x (8 bundles of 4x4)
    h2   = S S h0,  S = diag(a) A^T diag(a)   (2 propagation steps)
    h3   = inverse rotation of h2
    out  = GELU_exact(h3 @ w1.T + b1) @ w2.T + b2

Sharding: nodes partitioned contiguously across 8 cores (12500 each). Edges
bucketed by owning dst shard; per core, grouped by (dst tile of 128, src
chunk of 25088 replica rows) so gathers use int16 indices. The propagation
step is: dma_gather rows from the replicated node table, build one-hot
dst matrices on DVE (is_equal vs iota), accumulate with PE matmuls into
PSUM. AllGather replicates the node table between steps. The separable
norm coefs a[src]*a[dst] are folded into the stored node tables, never
per-edge.
"""
import os
import sys

sys.path.insert(0, "/opt/trn_rl_repo")

import numpy as np

N_NODES = 100000
N_EDGES = 1600000
DIM = 128
HID = 256
N_CORES = 8
NSH = 12500                 # nodes per shard
NSHP = 12544                # padded shard rows (98 * 128)
NT = NSHP // 128            # dst tiles per core = 98
NREP = NSHP * N_CORES       # replica table rows = 100352
CHUNK = NREP // 4           # gather chunk rows = 25088 (int16-addressable)
N_CHUNKS = 4
PAD_DST = 1000.0            # dst_local sentinel for padding edges

# module globals: last run state (test.py reuses these for timing)
LAST_RESULTS = None
LAST_NC = None
LAST_IN_MAPS = None


# ----------------------------------------------------------------- host prep

def _wrap_idx(idx_flat: np.ndarray) -> np.ndarray:
    """[n] -> [128, n/16] int16 wrapped+replicated dma_gather index layout."""
    w = idx_flat.reshape(-1, 16).T.astype(np.int16)
    return np.tile(w, (8, 1))


def preprocess(x, node_rep, src, dst, w1, b1, w2, b2):
    """Build per-core input maps + the static SPMD edge-group structure."""
    deg = np.bincount(src, minlength=N_NODES).astype(np.float64)
    a64 = 1.0 / np.sqrt(deg)
    a = a64.astype(np.float32)
    a2 = (1.0 / deg).astype(np.float32)

    # global node id -> replica-table row
    def rrow(u):
        return (u // NSH) * NSHP + (u % NSH)

    src_rrow = rrow(src)
    dst_core = dst // NSH

    # per (core, tile, chunk) bucket counts -> static max group structure
    counts = np.zeros((N_CORES, NT * N_CHUNKS), np.int64)
    per_core = []
    for c in range(N_CORES):
        m = dst_core == c
        dl = (dst[m] - c * NSH).astype(np.int64)      # local dst
        sr = src_rrow[m]
        tile_id = dl // 128
        chunk_id = sr // CHUNK
        key = tile_id * N_CHUNKS + chunk_id
        order = np.argsort(key, kind="stable")
        per_core.append((dl[order], sr[order], key[order]))
        counts[c] = np.bincount(key, minlength=NT * N_CHUNKS)

    G = np.ceil(counts.max(axis=0) / 128.0).astype(np.int64)  # [NT*4] groups
    g_off = np.concatenate([[0], np.cumsum(G)])               # group offsets
    total_groups = int(g_off[-1])
    total_edges_padded = total_groups * 128

    structure = G.reshape(NT, N_CHUNKS)

    iota = np.tile(np.arange(128, dtype=np.float32), (128, 1))
    ident = np.eye(128, dtype=np.float32)
    w1t = np.ascontiguousarray(w1.T)                       # [DIM, HID]
    b1h = np.ascontiguousarray(b1.reshape(2, 128).T)       # [128, 2]
    w2t2 = np.ascontiguousarray(w2.T.reshape(2, 128, DIM).transpose(1, 0, 2))
    b2c = np.ascontiguousarray(b2.reshape(128, 1))

    in_maps = []
    for c in range(N_CORES):
        dl, sr, key = per_core[c]
        n = dl.shape[0]
        # position of each edge inside the padded stream
        bucket_starts = g_off[:-1] * 128                    # [NT*4]
        within = np.arange(n) - np.concatenate(
            [[0], np.cumsum(np.bincount(key, minlength=NT * N_CHUNKS))]
        )[key]
        pos = bucket_starts[key] + within
        idx_pad = np.zeros(total_edges_padded, np.int64)
        dst_pad = np.full(total_edges_padded, PAD_DST, np.float32)
        idx_pad[pos] = sr % CHUNK
        dst_pad[pos] = (dl % 128).astype(np.float32)

        rows = slice(c * NSH, (c + 1) * NSH)
        x_sh = np.zeros((NSHP, DIM), np.float32)
        x_sh[:NSH] = x[rows]
        rep_sh = np.zeros((NSHP, DIM), np.float32)
        rep_sh[:NSH] = node_rep[rows].reshape(NSH, DIM)
        a_pad = np.zeros(NSHP, np.float32)
        a_pad[:NSH] = a[rows]
        a2_pad = np.zeros(NSHP, np.float32)
        a2_pad[:NSH] = a2[rows]

        in_maps.append({
            "x_sh": x_sh,
            "rep_sh": rep_sh,
            "idx_all": _wrap_idx(idx_pad),                       # [128, tg*8]
            "dst_all": np.ascontiguousarray(
                dst_pad.reshape(total_groups, 128).T),           # [128, tg]
            "a_col": np.ascontiguousarray(
                a_pad.reshape(NT, 128).T),                       # [128, NT]
            "a2_col": np.ascontiguousarray(
                a2_pad.reshape(NT, 128).T),                      # [128, NT]
            "iota": iota,
            "ident": ident,
            "w1t": w1t,
            "b1h": b1h,
            "w2t2": w2t2,
            "b2c": b2c,
        })
    return in_maps, structure, total_groups


# -------------------------------------------------------------- device build

def build_nc(structure, total_groups):
    import concourse.bacc as bacc
    import concourse.mybir as mybir
    import concourse.tile as tile

    f32 = mybir.dt.float32
    nc = bacc.Bacc("TRN2", target_bir_lowering=False, debug=False,
                   num_devices=N_CORES)

    x_sh = nc.dram_tensor("x_sh", [NSHP, DIM], f32, kind="ExternalInput")
    rep_sh = nc.dram_tensor("rep_sh", [NSHP, DIM], f32, kind="ExternalInput")
    idx_all = nc.dram_tensor("idx_all", [128, total_groups * 8],
                             mybir.dt.int16, kind="ExternalInput")
    dst_all = nc.dram_tensor("dst_all", [128, total_groups], f32,
                             kind="ExternalInput")
    a_col = nc.dram_tensor("a_col", [128, NT], f32, kind="ExternalInput")
    a2_col = nc.dram_tensor("a2_col", [128, NT], f32, kind="ExternalInput")
    iota = nc.dram_tensor("iota", [128, 128], f32, kind="ExternalInput")
    ident = nc.dram_tensor("ident", [128, 128], f32, kind="ExternalInput")
    w1t = nc.dram_tensor("w1t", [DIM, HID], f32, kind="ExternalInput")
    b1h = nc.dram_tensor("b1h", [128, 2], f32, kind="ExternalInput")
    w2t2 = nc.dram_tensor("w2t2", [128, 2, DIM], f32, kind="ExternalInput")
    b2c = nc.dram_tensor("b2c", [128, 1], f32, kind="ExternalInput")
    out_t = nc.dram_tensor("out_t", [DIM, NSHP], f32, kind="ExternalOutput")

    G = structure
    gmax = int(G.max())

    with tile.TileContext(nc) as tc:
        with (
            tc.tile_pool(name="const", bufs=1) as cp,
            tc.tile_pool(name="io", bufs=3) as iop,
            tc.tile_pool(name="rotp", bufs=2) as rotp,
            tc.tile_pool(name="gath", bufs=3) as gp,
            tc.tile_pool(name="m2", bufs=4) as m2p,
            tc.tile_pool(name="outp", bufs=3) as op,
            tc.tile_pool(name="psum", bufs=2, space="PSUM") as pp,
            tc.tile_pool(name="dram", bufs=1, space="DRAM") as dp,
        ):
            # ---- constants into SBUF
            iota_sb = cp.tile([128, 128], f32)
            nc.sync.dma_start(iota_sb[:], iota[:])
            id_sb = cp.tile([128, 128], f32)
            nc.sync.dma_start(id_sb[:], ident[:])
            idx_sb = cp.tile([128, total_groups * 8], mybir.dt.int16)
            nc.sync.dma_start(idx_sb[:], idx_all[:])
            dst_sb = cp.tile([128, total_groups], f32)
            nc.sync.dma_start(dst_sb[:], dst_all[:])
            a_sb = cp.tile([128, NT], f32)
            nc.sync.dma_start(a_sb[:], a_col[:])
            a2_sb = cp.tile([128, NT], f32)
            nc.sync.dma_start(a2_sb[:], a2_col[:])
            w1t_sb = cp.tile([DIM, HID], f32)
            nc.sync.dma_start(w1t_sb[:], w1t[:])
            b1h_sb = cp.tile([128, 2], f32)
            nc.sync.dma_start(b1h_sb[:], b1h[:])
            w2t2_sb = cp.tile([128, 2, DIM], f32)
            nc.sync.dma_start(w2t2_sb[:], w2t2[:])
            b2c_sb = cp.tile([128, 1], f32)
            nc.sync.dma_start(b2c_sb[:], b2c[:])

            g0_sh = dp.tile([NSHP, DIM], f32)
            g0_rep = dp.tile([NREP, DIM], f32)
            g1_sh = dp.tile([NSHP, DIM], f32)
            g1_rep = dp.tile([NREP, DIM], f32)

            def rotation(x_t, rep_t, dest, dest_tag, transposed):
                """dest = einsum(rep, x) per node; 8 bundles of 4x4."""
                x4 = x_t[:].rearrange("p (b d e) -> p b d e", b=8, d=4, e=4)
                r4 = rep_t[:].rearrange("p (b c d) -> p b c d", b=8, c=4, d=4)
                tmp = rotp.tile([128, DIM], f32, tag=dest_tag + "_tmp")
                for d in range(4):
                    if transposed:
                        # out[b,c,e] += rep[b,d,c] * x[b,d,e]
                        a_d = r4[:, :, d, :].unsqueeze(3)
                    else:
                        # out[b,c,e] += rep[b,c,d] * x[b,d,e]
                        a_d = r4[:, :, :, d].unsqueeze(3)
                    a_d = a_d.broadcast_to((128, 8, 4, 4))
                    b_d = x4[:, :, d, :].unsqueeze(2).broadcast_to((128, 8, 4, 4))
                    dst4 = (dest if d == 0 else tmp)[:].rearrange(
                        "p (b c e) -> p b c e", b=8, c=4, e=4)
                    nc.vector.tensor_tensor(dst4, a_d, b_d,
                                            op=mybir.AluOpType.mult)
                    if d > 0:
                        nc.vector.tensor_tensor(dest[:], dest[:], tmp[:],
                                                op=mybir.AluOpType.add)

            # ---- phase R1: g0 = rotate(x) * a
            for t in range(NT):
                x_t = iop.tile([128, DIM], f32, tag="x_t")
                nc.sync.dma_start(x_t[:], x_sh[t * 128:(t + 1) * 128, :])
                rep_t = iop.tile([128, DIM], f32, tag="rep_t")
                nc.sync.dma_start(rep_t[:], rep_sh[t * 128:(t + 1) * 128, :])
                rot = rotp.tile([128, DIM], f32, tag="rot")
                rotation(x_t, rep_t, rot, "rot", transposed=False)
                g0_t = op.tile([128, DIM], f32, tag="g0_t")
                nc.scalar.mul(g0_t[:], rot[:], a_sb[:, t:t + 1])
                nc.sync.dma_start(g0_sh[t * 128:(t + 1) * 128, :], g0_t[:])

            nc.gpsimd.collective_compute(
                "AllGather", mybir.AluOpType.bypass,
                ins=[g0_sh.opt()], outs=[g0_rep.opt()],
                replica_groups=[list(range(N_CORES))],
            )

            def prop_step(g_rep, scale_sb, out_cb):
                """One propagation: for each dst tile, gather + one-hot MM."""
                for t in range(NT):
                    acc = pp.tile([128, DIM], f32, tag="acc")
                    n_mm = int(sum(G[t]))
                    mm = 0
                    for k in range(N_CHUNKS):
                        gk = int(G[t][k])
                        if gk == 0:
                            continue
                        g0k = int(np.sum(G[:t]) + sum(G[t][:k]))
                        gth = gp.tile([128, gmax, DIM], f32, tag="gth")
                        nc.gpsimd.dma_gather(
                            gth[:, :gk, :],
                            g_rep[k * CHUNK:(k + 1) * CHUNK, :],
                            idx_sb[:, g0k * 8:(g0k + gk) * 8],
                            128 * gk, 128 * gk, DIM,
                        )
                        for j in range(gk):
                            m2 = m2p.tile([128, 128], f32, tag="m2")
                            nc.vector.tensor_scalar(
                                m2[:], iota_sb[:],
                                dst_sb[:, g0k + j:g0k + j + 1], None,
                                op0=mybir.AluOpType.is_equal,
                            )
                            nc.tensor.matmul(
                                acc[:], m2[:], gth[:, j, :],
                                start=(mm == 0), stop=(mm == n_mm - 1),
                            )
                            mm += 1
                    out_cb(t, acc, scale_sb)

            # ---- phase P1: g1 = A^T g0, scaled by a^2
            def p1_out(t, acc, scale_sb):
                g1_t = op.tile([128, DIM], f32, tag="g1_t")
                nc.scalar.mul(g1_t[:], acc[:], scale_sb[:, t:t + 1])
                nc.sync.dma_start(g1_sh[t * 128:(t + 1) * 128, :], g1_t[:])

            prop_step(g0_rep, a2_sb, p1_out)

            nc.gpsimd.collective_compute(
                "AllGather", mybir.AluOpType.bypass,
                ins=[g1_sh.opt()], outs=[g1_rep.opt()],
                replica_groups=[list(range(N_CORES))],
            )

            # ---- phase P2 + inverse rotation + FFN, fused per tile
            def p2_out(t, acc, scale_sb):
                h2 = rotp.tile([128, DIM], f32, tag="h2")
                nc.scalar.mul(h2[:], acc[:], scale_sb[:, t:t + 1])
                rep_t = iop.tile([128, DIM], f32, tag="rep2_t")
                nc.sync.dma_start(rep_t[:], rep_sh[t * 128:(t + 1) * 128, :])
                h3 = rotp.tile([128, DIM], f32, tag="h3")
                rotation(h2, rep_t, h3, "h3", transposed=True)
                # transpose to [feat, node]
                tp = pp.tile([128, 128], f32, tag="tp")
                nc.tensor.transpose(tp[:], h3[:], id_sb[:])
                h3t = rotp.tile([128, 128], f32, tag="h3t")
                nc.scalar.copy(h3t[:], tp[:])
                # FFN layer 1 + exact GELU
                act = rotp.tile([128, 2, 128], f32, tag="act")
                for h in range(2):
                    ps1 = pp.tile([128, 128], f32, tag="ps1")
                    nc.tensor.matmul(ps1[:], w1t_sb[:, h * 128:(h + 1) * 128],
                                     h3t[:], start=True, stop=True)
                    nc.scalar.activation(act[:, h, :], ps1[:],
                                         mybir.ActivationFunctionType.Gelu,
                                         bias=b1h_sb[:, h:h + 1])
                # FFN layer 2 + bias
                ps2 = pp.tile([128, 128], f32, tag="ps2")
                for h in range(2):
                    nc.tensor.matmul(ps2[:], w2t2_sb[:, h, :], act[:, h, :],
                                     start=(h == 0), stop=(h == 1))
                o_t = op.tile([128, 128], f32, tag="o_t")
                nc.scalar.activation(o_t[:], ps2[:],
                                     mybir.ActivationFunctionType.Identity,
                                     bias=b2c_sb[:])
                nc.sync.dma_start(out_t[:, t * 128:(t + 1) * 128], o_t[:])

            prop_step(g1_rep, a_sb, p2_out)

    nc.compile()
    return nc


# -------------------------------------------------------------------- runner

def kernel(x, node_rep, src, dst, w1, b1, w2, b2):
    global LAST_RESULTS, LAST_NC, LAST_IN_MAPS
    from concourse import bass_utils

    x = np.asarray(x, np.float32)
    node_rep = np.asarray(node_rep, np.float32)
    src = np.asarray(src, np.int64)
    dst = np.asarray(dst, np.int64)
    w1 = np.asarray(w1, np.float32)
    b1 = np.asarray(b1, np.float32)
    w2 = np.asarray(w2, np.float32)
    b2 = np.asarray(b2, np.float32)

    in_maps, structure, total_groups = preprocess(
        x, node_rep, src, dst, w1, b1, w2, b2)
    nc = build_nc(structure, total_groups)
    res = bass_utils.run_bass_kernel_spmd(
        nc, in_maps, core_ids=list(range(N_CORES)),
        trace=bool(os.environ.get("BASS_TRACE")),
    )
    LAST_RESULTS = res
    LAST_NC = nc
    LAST_IN_MAPS = in_maps
    out = np.concatenate(
        [res.results[c]["out_t"].T[:NSH] for c in range(N_CORES)], axis=0)
    return np.ascontiguousarray(out)
